# revision 8
# baseline (speedup 1.0000x reference)
"""Trainium2 Bass kernel for nn_EntanglementPropagator (gnn_message_passing).

Math: the reference computes, for edges e=(src[e], dst[e]):
    eff_w[e,f]   = W[s,d,f] * cos(phase[s,d])
    signal[b,e,f]= x[b,s,f] * eff_w[e,f]
    out[b,n,f]   = (sum_{e: dst[e]==n} signal[b,e,f]) / max(out_deg[n],1)

Folding edge multiplicity M[s,d] and the 1/norm factor into a per-(s,d)
scale C[s,d] = cos(phase[s,d])*M[s,d]/norm[d]:

    out[b,d,f] = sum_s (W[s,d,f] * C[s,d]) * x[b,s,f]

The contraction is elementwise in f, so sharding over f is traffic-optimal:
core c owns f in [c*32,(c+1)*32) and reads W[:,:,fsl] + x[:,:,fsl] exactly
once (no replication of the big W tensor, no collectives).  Inputs/outputs
are bf16 (measured end-to-end rel-err ~5e-3, inside the 2e-2 gate): per
core ~5.1 MB in + 0.5 MB out vs 16 MB/core for the previous dst-sharded
fp32 design.

Compute layout per core, per feature f: out_f[b,d] = x_f[s,b]^T @ W'_f[s,d]
with the small x_f stationary (LDWEIGHTS 32 cols, pull-ahead hides it) and
the big W'_f streamed (256 cols -> ~107ns/MM warm).  Features are processed
in groups of 3 packed into PSUM partition col-strips {0,32,64} via
tile_position (col 96 = quadrant 3 is broken in HW): col-tiled MMs run
CONCURRENTLY in the array (~2.4x), and drains run at 96-lane width on ACT,
converting fp32 psum -> bf16 on the fly.  The two source halves (kb)
accumulate in PSUM start/stop.  W'=W*C scaling runs on DVE in bf16 2x
packed mode (~1.14us per [128,8,256] piece) and paces the MM stream.

Trace-driven layout decisions (v3):
  * ALL input DMAs ride the sync ring in the order ph, ms, w(ch0), x,
    w(ch1..3): the ACT engine then starts its Sin table load + C chain
    immediately, so the DVE scale chain starts as soon as w(ch0) lands
    (was t=18.8us when ph/ms/x sat on the ACT ring behind sem batching).
  * ph/ms are host-pre-swizzled to [p, kb, d] so their DMAs are plain
    contiguous (2KB/1KB lines, 128 descriptors instead of 256).
  * All 4 W chunks stay live in SBUF (wpool bufs=4) - recycling caused a
    4.1us DMA stall waiting on MM consumers.
  * Drains collect 4 groups into one o_sb tile; out-DMAs (ACT ring, the
    only thing on it) then move 12 f-planes at a time with 2KB lines
    (was 11 DMAs with 512B lines, 1056 descriptors).
"""

import numpy as np
import ml_dtypes

import concourse.mybir as mybir
import concourse.tile as tile
from concourse import bacc
from concourse.bass_utils import run_bass_kernel_spmd

N = 256          # nodes
F = 256          # feature dim
B = 32           # batch
N_CORES = 8
FS = F // N_CORES        # f-planes per core = 32
KB = 2                   # source-node partition blocks (s: 2 x 128)
FCS = [8, 8, 8, 8]       # f-planes per W DMA chunk
FOFF = [0, 8, 16, 24]    # chunk start f
SSL = [4, 4, 4, 4]       # scale slice width (f-planes per DVE scale op)
NCH = len(FCS)
NG = len([3] * 10 + [2])
GROUPS = [3] * 10 + [2]  # f-planes per PSUM col-strip group (3*10+2 = 32)
BATCHES = [(0, 4), (4, 4), (8, 2), (10, 1)]  # (first group, n groups) per out DMA
F32 = mybir.dt.float32
BF16 = mybir.dt.bfloat16

HALF_PI = float(np.pi / 2.0)


def build_body(tc, w, aux, ph, out):
    """Emit one iteration of the kernel body.

    w   [N, FS, N]     DRAM bf16 - W[:, :, fsl] transposed to [s, f_loc, d]
    aux [128, 2560]    DRAM bf16 - x both kb halves [p, f, b] + ms, packed
    ph  [128, KB, N]   DRAM f32  - phase, host-swizzled to [s%128, s//128, d]
    out [96, NG, N]    DRAM bf16 - [(fi b), group, d]; host decodes f=3g+fi
    """
    nc = tc.nc

    with (
        tc.tile_pool(name="cpool", bufs=1) as cpool,
        tc.tile_pool(name="wpool", bufs=NCH) as wpool,
        tc.tile_pool(name="opool", bufs=2) as opool,
        tc.tile_pool(name="ppool", bufs=4, space="PSUM") as ppool,
    ):
        # --- input stream (sync ring): ph, ms, w chunk 0, x, w chunks 1..3.
        bias_t = cpool.tile([128, 1], F32, tag="bias")
        nc.vector.memset(bias_t, -HALF_PI)
        ph_t = cpool.tile([128, KB, N], F32, tag="ph")
        nc.sync.dma_start(out=ph_t, in_=ph)
        # aux packs x (both kb halves) + ms into one bf16 tensor with 5KB
        # lines: 128 descriptors instead of 384 (small descriptors pay
        # ~60ns/desc ring overhead and skew the per-dma completion).
        aux_t = cpool.tile([128, KB * FS * B + KB * N], BF16, tag="aux")
        nc.sync.dma_start(out=aux_t, in_=aux)
        xt = {kb: aux_t[:, kb * FS * B:(kb + 1) * FS * B]
              .rearrange("p (f b) -> p f b", b=B) for kb in range(KB)}
        ms_t = aux_t[:, KB * FS * B:].rearrange("p (k d) -> p k d", k=KB)
        wt = []
        for ch in range(NCH):
            fsl = slice(FOFF[ch], FOFF[ch] + FCS[ch])
            wch = {}
            for kb in range(KB):
                ssl = slice(kb * 128, (kb + 1) * 128)
                wch[kb] = wpool.tile([128, FCS[ch], N], BF16,
                                     name=f"wt{ch}_{kb}", tag=f"w{ch}{kb}")
                nc.sync.dma_start(out=wch[kb], in_=w[ssl, fsl, :])
            wt.append(wch)

        # --- per-(s,d) scale C = cos(phase) * M/norm, bf16, [s_p, kb, d].
        # Sin LUT is only accurate on ~[-pi,pi]; half-angle form
        # cos(x) = 2*sin^2(x/2 - pi/2) - 1.  A dummy Sin on the bias tile
        # hoists the 1.3us ACT_TABLE_LOAD ahead of the ph DMA completion.
        warm = cpool.tile([128, 1], F32, tag="warm")
        nc.scalar.activation(out=warm, in_=bias_t,
                             func=mybir.ActivationFunctionType.Sin,
                             bias=0.0, scale=1.0)
        c_bf = {}
        for kb in range(KB):
            c32 = cpool.tile([128, N], F32, tag=f"c32_{kb}")
            nc.scalar.activation(out=c32, in_=ph_t[:, kb, :],
                                 func=mybir.ActivationFunctionType.Sin,
                                 bias=bias_t, scale=0.5)
            nc.vector.tensor_mul(out=c32, in0=c32, in1=c32)
            nc.vector.tensor_scalar(out=c32, in0=c32, scalar1=2.0,
                                    scalar2=-1.0,
                                    op0=mybir.AluOpType.mult,
                                    op1=mybir.AluOpType.add)
            cb = cpool.tile([128, N], BF16, tag=f"cbf_{kb}")
            nc.vector.tensor_mul(out=cb, in0=c32, in1=ms_t[:, kb, :])
            c_bf[kb] = cb

        # --- W' = W * C on DVE (bf16 2x packed mode), in slices of SSL
        # f-planes so MMs start as soon as a slice is scaled.
        for ch in range(NCH):
            for kb in range(KB):
                for s0 in range(0, FCS[ch], SSL[ch]):
                    sw = min(SSL[ch], FCS[ch] - s0)
                    nc.vector.tensor_mul(
                        out=wt[ch][kb][:, s0:s0 + sw, :],
                        in0=wt[ch][kb][:, s0:s0 + sw, :],
                        in1=c_bf[kb][:, None, :].broadcast_to([128, sw, N]))

        # --- compute in groups of <=3 f-planes packed into PSUM col-strips
        # {0,32,64}; kb accumulates start/stop; ACT drains into a batched
        # o_sb, one wide out-DMA (ACT ring) per batch.
        f2ch = {}
        for ch in range(NCH):
            for fl in range(FOFF[ch], FOFF[ch] + FCS[ch]):
                f2ch[fl] = ch
        for b0, nb in BATCHES:
            gsizes = GROUPS[b0:b0 + nb]
            bf = sum(gsizes)
            f0b = sum(GROUPS[:b0])
            o_sb = opool.tile([96, nb, N], BF16, name="o_sb")
            f0 = f0b
            for sl, gsz in enumerate(gsizes):
                ps = ppool.tile([96, N], F32, name="ps")
                for kb in range(KB):
                    for fi in range(gsz):
                        fl = f0 + fi
                        nc.tensor.matmul(
                            ps[fi * 32:(fi + 1) * 32, :],
                            lhsT=xt[kb][:, fl, :],
                            rhs=wt[f2ch[fl]][kb][:, fl - FOFF[f2ch[fl]], :],
                            start=(kb == 0), stop=(kb == KB - 1))
                nc.scalar.copy(out=o_sb[:gsz * 32, sl, :],
                               in_=ps[:gsz * 32, :])
                f0 += gsz
            gsz = gsizes[0]   # uniform within a batch (3,3,3,3 / 2 / ...)
            assert all(g == gsz for g in gsizes)
            nc.scalar.dma_start(out=out[:gsz * 32, b0:b0 + nb, :],
                                in_=o_sb[:gsz * 32, :, :])


def build_program(n_repeat=1, loop_k=None):
    nc = bacc.Bacc("TRN2", target_bir_lowering=False, debug=False,
                   num_devices=N_CORES)
    w = nc.dram_tensor("w", [N, FS, N], BF16, kind="ExternalInput").ap()
    aux = nc.dram_tensor("aux", [128, KB * FS * B + KB * N], BF16,
                         kind="ExternalInput").ap()
    ph = nc.dram_tensor("ph", [128, KB, N], F32, kind="ExternalInput").ap()
    out = nc.dram_tensor("out", [96, NG, N], BF16, kind="ExternalOutput").ap()

    with tile.TileContext(nc) as tc:
        if loop_k is not None:
            with tc.For_i(0, loop_k, 1):
                for _ in range(n_repeat):
                    build_body(tc, w, aux, ph, out)
        else:
            for _ in range(n_repeat):
                build_body(tc, w, aux, ph, out)
    nc.compile()
    return nc


def host_prep(phase, src, dst):
    """Per-(s,d) multiplicity / out-degree normalization from the integer
    edge tensors.  Returns ms [N, N] float32 with ms[s,d] = M[s,d]/norm[d]."""
    src = np.asarray(src).astype(np.int64)
    dst = np.asarray(dst).astype(np.int64)
    counts = np.bincount(src, minlength=N).astype(np.float64)
    norm = np.maximum(counts, 1.0)
    mult = np.bincount(src * N + dst, minlength=N * N).astype(np.float64)
    mult = mult.reshape(N, N)
    ms = (mult / norm[None, :]).astype(np.float32)
    return ms


_PROGRAM_CACHE = {}


def get_program(n_repeat=1, loop_k=None):
    key = (n_repeat, loop_k)
    if key not in _PROGRAM_CACHE:
        _PROGRAM_CACHE[key] = build_program(n_repeat, loop_k)
    return _PROGRAM_CACHE[key]


def make_in_maps(node_features, W, phase, src, dst):
    bf16 = ml_dtypes.bfloat16
    node_features = np.asarray(node_features, dtype=np.float32)
    W = np.asarray(W, dtype=np.float32)
    phase = np.asarray(phase, dtype=np.float32)
    msf = host_prep(phase, src, dst)
    # swizzle [s, d] -> [s%128, s//128, d] so the DMA is plain contiguous
    ph_sw = np.ascontiguousarray(
        phase.reshape(KB, 128, N).transpose(1, 0, 2))
    ms_sw = msf.reshape(KB, 128, N).transpose(1, 0, 2).astype(bf16)
    # f-major layouts (module docstring): transposes + bf16 cast, no math.
    WT = W.astype(bf16).transpose(0, 2, 1)              # [s, f, d] view
    xT = node_features.astype(bf16).transpose(1, 2, 0)  # [s, f, b] view
    in_maps = []
    for c in range(N_CORES):
        fsl = slice(c * FS, (c + 1) * FS)
        xc = xT[:, fsl, :].reshape(KB, 128, FS * B)     # [kb, p, f*b]
        aux = np.concatenate(
            [xc[0], xc[1], ms_sw.reshape(128, KB * N)], axis=1)
        in_maps.append({
            "w": np.ascontiguousarray(WT[:, fsl, :]),
            "aux": np.ascontiguousarray(aux),
            "ph": ph_sw,
        })
    return in_maps


def kernel(node_features, W, phase, src, dst):
    nc = get_program(1)
    in_maps = make_in_maps(node_features, W, phase, src, dst)
    res = run_bass_kernel_spmd(nc, in_maps, list(range(N_CORES)))
    arr = np.stack([res.results[c]["out"] for c in range(N_CORES)], axis=0)
    # [c, (fi b), g, d] bf16 -> [b, d, f] f32 with f = c*FS + 3*g + fi
    arr = arr.astype(np.float32).reshape(N_CORES, 3, B, NG, N)
    out = arr.transpose(2, 4, 0, 3, 1).reshape(B, N, N_CORES, 3 * NG)
    return np.ascontiguousarray(out[:, :, :, :FS].reshape(B, N, F))


# revision 9
# speedup vs baseline: 1.0851x; 1.0851x over previous
"""Trainium2 Bass kernel for nn_EntanglementPropagator (gnn_message_passing).

Math: the reference computes, for edges e=(src[e], dst[e]):
    eff_w[e,f]   = W[s,d,f] * cos(phase[s,d])
    signal[b,e,f]= x[b,s,f] * eff_w[e,f]
    out[b,n,f]   = (sum_{e: dst[e]==n} signal[b,e,f]) / max(out_deg[n],1)

Folding edge multiplicity M[s,d] and the 1/norm factor into a per-(s,d)
scale C[s,d] = cos(phase[s,d])*M[s,d]/norm[d]:

    out[b,d,f] = sum_s (W[s,d,f] * C[s,d]) * x[b,s,f]

The contraction is elementwise in f, so sharding over f is traffic-optimal:
core c owns f in [c*32,(c+1)*32) and reads W[:,:,fsl] + x[:,:,fsl] exactly
once (no replication of the big W tensor, no collectives).  Inputs/outputs
are bf16 (measured end-to-end rel-err ~5e-3, inside the 2e-2 gate): per
core ~5.1 MB in + 0.5 MB out vs 16 MB/core for the previous dst-sharded
fp32 design.

Compute layout per core, per feature f: out_f[b,d] = x_f[s,b]^T @ W'_f[s,d]
with the small x_f stationary (LDWEIGHTS 32 cols, pull-ahead hides it) and
the big W'_f streamed (256 cols -> ~107ns/MM warm).  Features are processed
in groups of 3 packed into PSUM partition col-strips {0,32,64} via
tile_position (col 96 = quadrant 3 is broken in HW): col-tiled MMs run
CONCURRENTLY in the array (~2.4x), and drains run at 96-lane width on ACT,
converting fp32 psum -> bf16 on the fly.  The two source halves (kb)
accumulate in PSUM start/stop.  W'=W*C scaling runs on DVE in bf16 2x
packed mode (~1.14us per [128,8,256] piece) and paces the MM stream.

Trace-driven layout decisions (v3):
  * ALL input DMAs ride the sync ring in the order ph, ms, w(ch0), x,
    w(ch1..3): the ACT engine then starts its Sin table load + C chain
    immediately, so the DVE scale chain starts as soon as w(ch0) lands
    (was t=18.8us when ph/ms/x sat on the ACT ring behind sem batching).
  * ph/ms are host-pre-swizzled to [p, kb, d] so their DMAs are plain
    contiguous (2KB/1KB lines, 128 descriptors instead of 256).
  * All 4 W chunks stay live in SBUF (wpool bufs=4) - recycling caused a
    4.1us DMA stall waiting on MM consumers.
  * Drains collect 4 groups into one o_sb tile; out-DMAs (ACT ring, the
    only thing on it) then move 12 f-planes at a time with 2KB lines
    (was 11 DMAs with 512B lines, 1056 descriptors).
"""

import numpy as np
import ml_dtypes

import concourse.mybir as mybir
import concourse.tile as tile
from concourse import bacc
from concourse.bass_utils import run_bass_kernel_spmd

N = 256          # nodes
F = 256          # feature dim
B = 32           # batch
N_CORES = 8
FS = F // N_CORES        # f-planes per core = 32
KB = 2                   # source-node partition blocks (s: 2 x 128)
FCS = [8, 8, 8, 6, 2]    # f-planes per W DMA chunk; the tail chunks are
                         # small and align exactly with PSUM groups 8-9 and
                         # 10, shortening the serial scale->MM->drain->DMA
                         # chain after the last DMA byte lands
FOFF = [0, 8, 16, 24, 30]  # chunk start f
SSL = [4, 4, 4, 3, 2]    # scale slice width (f-planes per DVE scale op)
NCH = len(FCS)
NG = len([3] * 10 + [2])
GROUPS = [3] * 10 + [2]  # f-planes per PSUM col-strip group (3*10+2 = 32)
BATCHES = [(0, 4), (4, 4), (8, 2), (10, 1)]  # (first group, n groups) per out DMA
F32 = mybir.dt.float32
BF16 = mybir.dt.bfloat16

HALF_PI = float(np.pi / 2.0)


def build_body(tc, w, aux, ph, out):
    """Emit one iteration of the kernel body.

    w   [N, FS, N]     DRAM bf16 - W[:, :, fsl] transposed to [s, f_loc, d]
    aux [128, 2560]    DRAM bf16 - x both kb halves [p, f, b] + ms, packed
    ph  [128, KB, N]   DRAM f32  - phase, host-swizzled to [s%128, s//128, d]
    out [96, NG, N]    DRAM bf16 - [(fi b), group, d]; host decodes f=3g+fi
    """
    nc = tc.nc

    with (
        tc.tile_pool(name="cpool", bufs=1) as cpool,
        tc.tile_pool(name="wpool", bufs=NCH) as wpool,
        tc.tile_pool(name="opool", bufs=2) as opool,
        tc.tile_pool(name="ppool", bufs=4, space="PSUM") as ppool,
    ):
        # --- input stream (sync ring): ph, ms, w chunk 0, x, w chunks 1..3.
        bias_t = cpool.tile([128, 1], F32, tag="bias")
        nc.vector.memset(bias_t, -HALF_PI)
        ph_t = cpool.tile([128, KB, N], F32, tag="ph")
        nc.sync.dma_start(out=ph_t, in_=ph)
        # aux packs x (both kb halves) + ms into one bf16 tensor with 5KB
        # lines: 128 descriptors instead of 384 (small descriptors pay
        # ~60ns/desc ring overhead and skew the per-dma completion).
        aux_t = cpool.tile([128, KB * FS * B + KB * N], BF16, tag="aux")
        nc.sync.dma_start(out=aux_t, in_=aux)
        xt = {kb: aux_t[:, kb * FS * B:(kb + 1) * FS * B]
              .rearrange("p (f b) -> p f b", b=B) for kb in range(KB)}
        ms_t = aux_t[:, KB * FS * B:].rearrange("p (k d) -> p k d", k=KB)
        wt = []
        for ch in range(NCH):
            fsl = slice(FOFF[ch], FOFF[ch] + FCS[ch])
            wch = {}
            for kb in range(KB):
                ssl = slice(kb * 128, (kb + 1) * 128)
                wch[kb] = wpool.tile([128, FCS[ch], N], BF16,
                                     name=f"wt{ch}_{kb}", tag=f"w{ch}{kb}")
                nc.sync.dma_start(out=wch[kb], in_=w[ssl, fsl, :])
            wt.append(wch)

        # --- per-(s,d) scale C = cos(phase) * M/norm, bf16, [s_p, kb, d].
        # Sin LUT is only accurate on ~[-pi,pi]; half-angle form
        # cos(x) = 2*sin^2(x/2 - pi/2) - 1.  A dummy Sin on the bias tile
        # hoists the 1.3us ACT_TABLE_LOAD ahead of the ph DMA completion.
        warm = cpool.tile([128, 1], F32, tag="warm")
        nc.scalar.activation(out=warm, in_=bias_t,
                             func=mybir.ActivationFunctionType.Sin,
                             bias=0.0, scale=1.0)
        c_bf = {}
        for kb in range(KB):
            c32 = cpool.tile([128, N], F32, tag=f"c32_{kb}")
            nc.scalar.activation(out=c32, in_=ph_t[:, kb, :],
                                 func=mybir.ActivationFunctionType.Sin,
                                 bias=bias_t, scale=0.5)
            nc.vector.tensor_mul(out=c32, in0=c32, in1=c32)
            nc.vector.tensor_scalar(out=c32, in0=c32, scalar1=2.0,
                                    scalar2=-1.0,
                                    op0=mybir.AluOpType.mult,
                                    op1=mybir.AluOpType.add)
            cb = cpool.tile([128, N], BF16, tag=f"cbf_{kb}")
            nc.vector.tensor_mul(out=cb, in0=c32, in1=ms_t[:, kb, :])
            c_bf[kb] = cb

        # --- W' = W * C on DVE (bf16 2x packed mode), in slices of SSL
        # f-planes so MMs start as soon as a slice is scaled.
        for ch in range(NCH):
            for kb in range(KB):
                for s0 in range(0, FCS[ch], SSL[ch]):
                    sw = min(SSL[ch], FCS[ch] - s0)
                    nc.vector.tensor_mul(
                        out=wt[ch][kb][:, s0:s0 + sw, :],
                        in0=wt[ch][kb][:, s0:s0 + sw, :],
                        in1=c_bf[kb][:, None, :].broadcast_to([128, sw, N]))

        # --- compute in groups of <=3 f-planes packed into PSUM col-strips
        # {0,32,64}; kb accumulates start/stop; ACT drains into a batched
        # o_sb, one wide out-DMA (ACT ring) per batch.
        f2ch = {}
        for ch in range(NCH):
            for fl in range(FOFF[ch], FOFF[ch] + FCS[ch]):
                f2ch[fl] = ch
        for b0, nb in BATCHES:
            gsizes = GROUPS[b0:b0 + nb]
            bf = sum(gsizes)
            f0b = sum(GROUPS[:b0])
            o_sb = opool.tile([96, nb, N], BF16, name="o_sb")
            f0 = f0b
            for sl, gsz in enumerate(gsizes):
                ps = ppool.tile([96, N], F32, name="ps")
                for kb in range(KB):
                    for fi in range(gsz):
                        fl = f0 + fi
                        nc.tensor.matmul(
                            ps[fi * 32:(fi + 1) * 32, :],
                            lhsT=xt[kb][:, fl, :],
                            rhs=wt[f2ch[fl]][kb][:, fl - FOFF[f2ch[fl]], :],
                            start=(kb == 0), stop=(kb == KB - 1))
                nc.scalar.copy(out=o_sb[:gsz * 32, sl, :],
                               in_=ps[:gsz * 32, :])
                f0 += gsz
            gsz = gsizes[0]   # uniform within a batch (3,3,3,3 / 2 / ...)
            assert all(g == gsz for g in gsizes)
            nc.scalar.dma_start(out=out[:gsz * 32, b0:b0 + nb, :],
                                in_=o_sb[:gsz * 32, :, :])


def build_program(n_repeat=1, loop_k=None):
    nc = bacc.Bacc("TRN2", target_bir_lowering=False, debug=False,
                   num_devices=N_CORES)
    w = nc.dram_tensor("w", [N, FS, N], BF16, kind="ExternalInput").ap()
    aux = nc.dram_tensor("aux", [128, KB * FS * B + KB * N], BF16,
                         kind="ExternalInput").ap()
    ph = nc.dram_tensor("ph", [128, KB, N], F32, kind="ExternalInput").ap()
    out = nc.dram_tensor("out", [96, NG, N], BF16, kind="ExternalOutput").ap()

    with tile.TileContext(nc) as tc:
        if loop_k is not None:
            with tc.For_i(0, loop_k, 1):
                for _ in range(n_repeat):
                    build_body(tc, w, aux, ph, out)
        else:
            for _ in range(n_repeat):
                build_body(tc, w, aux, ph, out)
    nc.compile()
    return nc


def host_prep(phase, src, dst):
    """Per-(s,d) multiplicity / out-degree normalization from the integer
    edge tensors.  Returns ms [N, N] float32 with ms[s,d] = M[s,d]/norm[d]."""
    src = np.asarray(src).astype(np.int64)
    dst = np.asarray(dst).astype(np.int64)
    counts = np.bincount(src, minlength=N).astype(np.float64)
    norm = np.maximum(counts, 1.0)
    mult = np.bincount(src * N + dst, minlength=N * N).astype(np.float64)
    mult = mult.reshape(N, N)
    ms = (mult / norm[None, :]).astype(np.float32)
    return ms


_PROGRAM_CACHE = {}


def get_program(n_repeat=1, loop_k=None):
    key = (n_repeat, loop_k)
    if key not in _PROGRAM_CACHE:
        _PROGRAM_CACHE[key] = build_program(n_repeat, loop_k)
    return _PROGRAM_CACHE[key]


def make_in_maps(node_features, W, phase, src, dst):
    bf16 = ml_dtypes.bfloat16
    node_features = np.asarray(node_features, dtype=np.float32)
    W = np.asarray(W, dtype=np.float32)
    phase = np.asarray(phase, dtype=np.float32)
    msf = host_prep(phase, src, dst)
    # swizzle [s, d] -> [s%128, s//128, d] so the DMA is plain contiguous
    ph_sw = np.ascontiguousarray(
        phase.reshape(KB, 128, N).transpose(1, 0, 2))
    ms_sw = msf.reshape(KB, 128, N).transpose(1, 0, 2).astype(bf16)
    # f-major layouts (module docstring): transposes + bf16 cast, no math.
    WT = W.astype(bf16).transpose(0, 2, 1)              # [s, f, d] view
    xT = node_features.astype(bf16).transpose(1, 2, 0)  # [s, f, b] view
    in_maps = []
    for c in range(N_CORES):
        fsl = slice(c * FS, (c + 1) * FS)
        xc = xT[:, fsl, :].reshape(KB, 128, FS * B)     # [kb, p, f*b]
        aux = np.concatenate(
            [xc[0], xc[1], ms_sw.reshape(128, KB * N)], axis=1)
        in_maps.append({
            "w": np.ascontiguousarray(WT[:, fsl, :]),
            "aux": np.ascontiguousarray(aux),
            "ph": ph_sw,
        })
    return in_maps


def kernel(node_features, W, phase, src, dst):
    nc = get_program(1)
    in_maps = make_in_maps(node_features, W, phase, src, dst)
    res = run_bass_kernel_spmd(nc, in_maps, list(range(N_CORES)))
    arr = np.stack([res.results[c]["out"] for c in range(N_CORES)], axis=0)
    # [c, (fi b), g, d] bf16 -> [b, d, f] f32 with f = c*FS + 3*g + fi
    arr = arr.astype(np.float32).reshape(N_CORES, 3, B, NG, N)
    out = arr.transpose(2, 4, 0, 3, 1).reshape(B, N, N_CORES, 3 * NG)
    return np.ascontiguousarray(out[:, :, :, :FS].reshape(B, N, F))
